# revision 4
# baseline (speedup 1.0000x reference)
"""Trainium2 Bass kernel for nn_IrrepsIndexedLinear (segmented irrep linear).

Math: rows (8192) are grouped into 64 contiguous segments (sizes in
num_index_counts); segment g uses weight row w[g].  Per irrep (mul, ir):
  y[n, o, i] = coeff_k * sum_m x[n, m, i] * W_g[m, o],  W_g = w[g, off:off+mul^2].reshape(mul, mul)
  coeff_k = 1 / sqrt(64) / sqrt(mul)

Strategy (8 NeuronCores, SPMD single program):
- Row-shard: core c owns rows [1024c, 1024(c+1)).
- Host splits each core's rows into segment-pure chunks of <=128 rows, padded
  to exactly 128 (zeros), NCH chunks per core (max over cores, dummy-padded).
- Host packs inputs TRANSPOSED (contraction dim m on partitions) so all device
  DMAs are contiguous, 128-partition:
    in0[m, k*128 + r]                      = x0[row, m]
    in1[(k%2)*64 + m, (k//2)*384 + i*128 + r] = x1[row, m, i]   (pairs share cols)
    in2[(k%4)*32 + m, (k//4)*640 + i*128 + r] = x2[row, m, i]   (quads share cols)
  Weights gathered per chunk, laid out as lhsT (m, o) at matching partitions.
- Device: per chunk: matmul irrep0 (K=128,M=128,N=128), irrep1 (K=64,M=64,
  N=384, tile_position parity packing), irrep2 (K=32,M=32,N=512+128, quad
  packing).  PSUM evicted with copy*coeff to packed SBUF out images, DMA'd out.
- Host unpacks (inverse transpose) to the reference output layout.
"""

import math

import numpy as np

IRREPS = [(128, 1), (64, 3), (32, 5)]
NUM_IDX = 64
N_EL = 8192
NCORES = 8
RPC = N_EL // NCORES  # rows per core
C = 128  # chunk rows
W_OFF = [0, 128 * 128, 128 * 128 + 64 * 64]
COEFF = [1.0 / (8.0 * math.sqrt(128.0)),
         1.0 / (8.0 * math.sqrt(64.0)),
         1.0 / (8.0 * math.sqrt(32.0))]


def _plan(counts):
    """Per-core list of (segment, src_row, real_len) chunks, padded to common NCH."""
    starts = np.zeros(NUM_IDX + 1, np.int64)
    starts[1:] = np.cumsum(counts.astype(np.int64))
    assert starts[-1] == N_EL, f"counts sum {starts[-1]} != {N_EL}"
    per_core = []
    for c in range(NCORES):
        lo_c, hi_c = c * RPC, (c + 1) * RPC
        chunks = []
        for g in range(NUM_IDX):
            a = max(int(starts[g]), lo_c)
            b = min(int(starts[g + 1]), hi_c)
            if a >= b:
                continue
            for s in range(a, b, C):
                chunks.append((g, s, min(C, b - s)))
        per_core.append(chunks)
    nch = max(len(ch) for ch in per_core)
    nch = max(nch, 1)
    for ch in per_core:
        ch.extend([(0, 0, 0)] * (nch - len(ch)))
    return per_core, nch


def _pack_core(chunks, nch, x0, x1, x2, w):
    """Build the six packed device input arrays for one core."""
    p1 = (nch + 1) // 2
    p2 = (nch + 3) // 4
    in0 = np.zeros((128, nch * 128), np.float32)
    in1 = np.zeros((128, p1 * 384), np.float32)
    in2 = np.zeros((128, p2 * 640), np.float32)
    w0 = np.zeros((128, nch * 128), np.float32)
    w1 = np.zeros((128, p1 * 64), np.float32)
    w2 = np.zeros((128, p2 * 32), np.float32)
    for k, (g, s, ln) in enumerate(chunks):
        if ln > 0:
            in0[:, k * 128:k * 128 + ln] = x0[s:s + ln, :, 0].T
            # x1 block: (ln, 64, 3) -> (m, i, r)
            b1 = x1[s:s + ln].transpose(1, 2, 0)
            h = (k % 2) * 64
            c1 = (k // 2) * 384
            in1[h:h + 64, c1:c1 + 384].reshape(64, 3, 128)[:, :, :ln] = b1
            b2 = x2[s:s + ln].transpose(1, 2, 0)
            h2 = (k % 4) * 32
            c2 = (k // 4) * 640
            in2[h2:h2 + 32, c2:c2 + 640].reshape(32, 5, 128)[:, :, :ln] = b2
            w0[:, k * 128:(k + 1) * 128] = w[g, W_OFF[0]:W_OFF[0] + 128 * 128].reshape(128, 128)
            w1[(k % 2) * 64:(k % 2) * 64 + 64, (k // 2) * 64:(k // 2) * 64 + 64] = \
                w[g, W_OFF[1]:W_OFF[1] + 64 * 64].reshape(64, 64)
            w2[(k % 4) * 32:(k % 4) * 32 + 32, (k // 4) * 32:(k // 4) * 32 + 32] = \
                w[g, W_OFF[2]:W_OFF[2] + 32 * 32].reshape(32, 32)
    return {"in0": in0, "in1": in1, "in2": in2, "w0": w0, "w1": w1, "w2": w2}


def _unpack_core(chunks, outs, y0, y1, y2):
    out0, out1, out2 = outs["out0"], outs["out1"], outs["out2"]
    for k, (g, s, ln) in enumerate(chunks):
        if ln <= 0:
            continue
        y0[s:s + ln, :, 0] = out0[:, k * 128:k * 128 + ln].T
        h = (k % 2) * 64
        c1 = (k // 2) * 384
        # out1[h+o, c1 + i*128 + r] -> y1[s+r, o, i]
        blk = out1[h:h + 64, c1:c1 + 384].reshape(64, 3, 128)[:, :, :ln]
        y1[s:s + ln] = blk.transpose(2, 0, 1)
        h2 = (k % 4) * 32
        c2 = (k // 4) * 640
        blk2 = out2[h2:h2 + 32, c2:c2 + 640].reshape(32, 5, 128)[:, :, :ln]
        y2[s:s + ln] = blk2.transpose(2, 0, 1)


def _emulate_core(ins, nch):
    """Numpy emulation of the device program (for logic validation)."""
    p1 = (nch + 1) // 2
    p2 = (nch + 3) // 4
    out0 = np.zeros((128, nch * 128), np.float32)
    out1 = np.zeros((128, p1 * 384), np.float32)
    out2 = np.zeros((128, p2 * 640), np.float32)
    for k in range(nch):
        # irrep0
        lhsT = ins["w0"][:, k * 128:(k + 1) * 128]
        rhs = ins["in0"][:, k * 128:(k + 1) * 128]
        out0[:, k * 128:(k + 1) * 128] = COEFF[0] * (lhsT.T @ rhs)
        # irrep1
        h = (k % 2) * 64
        c1p = (k // 2)
        lhsT = ins["w1"][h:h + 64, c1p * 64:(c1p + 1) * 64]
        rhs = ins["in1"][h:h + 64, c1p * 384:(c1p + 1) * 384]
        out1[h:h + 64, c1p * 384:(c1p + 1) * 384] = COEFF[1] * (lhsT.T @ rhs)
        # irrep2
        h2 = (k % 4) * 32
        c2p = (k // 4)
        lhsT = ins["w2"][h2:h2 + 32, c2p * 32:(c2p + 1) * 32]
        rhs = ins["in2"][h2:h2 + 32, c2p * 640:(c2p + 1) * 640]
        out2[h2:h2 + 32, c2p * 640:(c2p + 1) * 640] = COEFF[2] * (lhsT.T @ rhs)
    return {"out0": out0, "out1": out1, "out2": out2}


# ---------------------------------------------------------------- device ---

_PROGRAM_CACHE = {}
LAST_RESULTS = None  # BassKernelResults of the most recent device run


def _build_program(nch):
    import concourse.mybir as mybir
    import concourse.tile as tile
    from concourse import bacc

    f32 = mybir.dt.float32
    p1 = (nch + 1) // 2
    p2 = (nch + 3) // 4

    nc = bacc.Bacc(None, target_bir_lowering=False)
    d_in0 = nc.dram_tensor("in0", [128, nch * 128], f32, kind="ExternalInput")
    d_in1 = nc.dram_tensor("in1", [128, p1 * 384], f32, kind="ExternalInput")
    d_in2 = nc.dram_tensor("in2", [128, p2 * 640], f32, kind="ExternalInput")
    d_w0 = nc.dram_tensor("w0", [128, nch * 128], f32, kind="ExternalInput")
    d_w1 = nc.dram_tensor("w1", [128, p1 * 64], f32, kind="ExternalInput")
    d_w2 = nc.dram_tensor("w2", [128, p2 * 32], f32, kind="ExternalInput")
    d_out0 = nc.dram_tensor("out0", [128, nch * 128], f32, kind="ExternalOutput")
    d_out1 = nc.dram_tensor("out1", [128, p1 * 384], f32, kind="ExternalOutput")
    d_out2 = nc.dram_tensor("out2", [128, p2 * 640], f32, kind="ExternalOutput")

    n_pieces = p2  # one piece = up to 4 chunks (2 pairs, 1 quad)

    def piece_chunks(q):
        return range(q * 4, min((q + 1) * 4, nch))

    with tile.TileContext(nc) as tc:
        with (
            tc.tile_pool(name="sb_in", bufs=1) as sb_in,
            tc.tile_pool(name="sb_out", bufs=1) as sb_out,
            tc.tile_pool(name="psA", bufs=2, space="PSUM") as psA,
        ):
            t_in0, t_in1, t_in2, t_w0, t_w1, t_w2 = [], [], [], [], [], []
            # input DMAs, piece-interleaved so early pieces complete first
            for q in range(n_pieces):
                ch = list(piece_chunks(q))
                nch_q = len(ch)
                npair_q = (nch_q + 1) // 2
                a0 = sb_in.tile([128, nch_q * 128], f32, tag=f"in0_{q}")
                nc.sync.dma_start(a0[:], d_in0[:, q * 512:q * 512 + nch_q * 128])
                a1 = sb_in.tile([128, npair_q * 384], f32, tag=f"in1_{q}")
                nc.sync.dma_start(a1[:], d_in1[:, q * 768:q * 768 + npair_q * 384])
                a2 = sb_in.tile([128, 640], f32, tag=f"in2_{q}")
                nc.sync.dma_start(a2[:], d_in2[:, q * 640:(q + 1) * 640])
                b0 = sb_in.tile([128, nch_q * 128], f32, tag=f"w0_{q}")
                nc.sync.dma_start(b0[:], d_w0[:, q * 512:q * 512 + nch_q * 128])
                b1 = sb_in.tile([128, npair_q * 64], f32, tag=f"w1_{q}")
                nc.sync.dma_start(b1[:], d_w1[:, q * 128:q * 128 + npair_q * 64])
                b2 = sb_in.tile([128, 32], f32, tag=f"w2_{q}")
                nc.sync.dma_start(b2[:], d_w2[:, q * 32:(q + 1) * 32])
                t_in0.append(a0); t_in1.append(a1); t_in2.append(a2)
                t_w0.append(b0); t_w1.append(b1); t_w2.append(b2)

            for q in range(n_pieces):
                ch = list(piece_chunks(q))
                nch_q = len(ch)
                npair_q = (nch_q + 1) // 2
                # 4 PSUM banks per piece (x2 bufs = all 8 banks):
                #   po0:   irrep0, 4 chunks x 128 cols
                #   po1a:  irrep1 pair0 [0:384] + irrep2 tail cols [384:512]
                #   po1b:  irrep1 pair1
                #   po2a:  irrep2 head (5 i-slices x 128 = 640 -> 512 here)
                po0 = psA.tile([128, nch_q * 128], f32, tag="po0")
                po1a = psA.tile([128, 512], f32, tag="po1a")
                po1 = [po1a[:, 0:384]]
                if npair_q > 1:
                    po1b = psA.tile([128, 384], f32, tag="po1b")
                    po1.append(po1b[:, :])
                po2a = psA.tile([128, 512], f32, tag="po2a")
                po2b = po1a[:, 384:512]
                for j, k in enumerate(ch):
                    # irrep0: K=128, M=128, N=128
                    nc.tensor.matmul(
                        po0[:, j * 128:(j + 1) * 128],
                        t_w0[q][:, j * 128:(j + 1) * 128],
                        t_in0[q][:, j * 128:(j + 1) * 128],
                        start=True, stop=True,
                    )
                    # irrep1: K=64, M=64, N=384 at partition parity
                    h = (k % 2) * 64
                    jp = j // 2
                    nc.tensor.matmul(
                        po1[jp][h:h + 64, :],
                        t_w1[q][h:h + 64, jp * 64:(jp + 1) * 64],
                        t_in1[q][h:h + 64, jp * 384:(jp + 1) * 384],
                        start=True, stop=True,
                        tile_position=(h, h),
                    )
                    # irrep2: K=32, M=32, N=512+128 at quad position
                    h2 = (k % 4) * 32
                    nc.tensor.matmul(
                        po2a[h2:h2 + 32, :],
                        t_w2[q][h2:h2 + 32, :],
                        t_in2[q][h2:h2 + 32, 0:512],
                        start=True, stop=True,
                        tile_position=(h2, h2),
                    )
                    nc.tensor.matmul(
                        po2b[h2:h2 + 32, :],
                        t_w2[q][h2:h2 + 32, :],
                        t_in2[q][h2:h2 + 32, 512:640],
                        start=True, stop=True,
                        tile_position=(h2, h2),
                    )
                # evictions (copy * coeff) into packed out tiles
                o0 = sb_out.tile([128, nch_q * 128], f32, tag="o0")
                nc.any.tensor_scalar_mul(o0[:], po0[:], COEFF[0])
                o1 = sb_out.tile([128, npair_q * 384], f32, tag="o1")
                for jp in range(npair_q):
                    nc.any.tensor_scalar_mul(
                        o1[:, jp * 384:(jp + 1) * 384], po1[jp][:], COEFF[1])
                o2 = sb_out.tile([128, 640], f32, tag="o2")
                nc.any.tensor_scalar_mul(o2[:, 0:512], po2a[:], COEFF[2])
                nc.any.tensor_scalar_mul(o2[:, 512:640], po2b[:], COEFF[2])
                nc.sync.dma_start(d_out0[:, q * 512:q * 512 + nch_q * 128], o0[:])
                nc.sync.dma_start(d_out1[:, q * 768:q * 768 + npair_q * 384], o1[:])
                nc.sync.dma_start(d_out2[:, q * 640:(q + 1) * 640], o2[:])

    nc.compile()
    return nc


def _run_device(per_core_ins, nch):
    global LAST_RESULTS
    from concourse.bass_utils import run_bass_kernel_spmd

    if nch not in _PROGRAM_CACHE:
        _PROGRAM_CACHE[nch] = _build_program(nch)
    nc = _PROGRAM_CACHE[nch]
    res = run_bass_kernel_spmd(nc, per_core_ins, core_ids=list(range(NCORES)))
    LAST_RESULTS = res
    return res.results


def kernel(x0, x1, x2, w, num_index_counts, _emulate=False):
    x0 = np.asarray(x0, np.float32)
    x1 = np.asarray(x1, np.float32)
    x2 = np.asarray(x2, np.float32)
    w = np.asarray(w, np.float32)
    counts = np.asarray(num_index_counts)
    per_core, nch = _plan(counts)
    per_core_ins = [_pack_core(ch, nch, x0, x1, x2, w) for ch in per_core]
    if _emulate:
        per_core_outs = [_emulate_core(ins, nch) for ins in per_core_ins]
    else:
        per_core_outs = _run_device(per_core_ins, nch)
    y0 = np.zeros((N_EL, 128, 1), np.float32)
    y1 = np.zeros((N_EL, 64, 3), np.float32)
    y2 = np.zeros((N_EL, 32, 5), np.float32)
    for ch, outs in zip(per_core, per_core_outs):
        _unpack_core(ch, outs, y0, y1, y2)
    return y0, y1, y2


# revision 9
# speedup vs baseline: 2.1574x; 2.1574x over previous
"""Trainium2 Bass kernel for nn_IrrepsIndexedLinear (segmented irrep linear).

Math: rows (8192) are grouped into 64 contiguous segments (sizes in
num_index_counts); segment g uses weight row w[g].  Per irrep (mul, ir):
  y[n, o, i] = coeff_k * sum_m x[n, m, i] * W_g[m, o],  W_g = w[g, off:off+mul^2].reshape(mul, mul)
  coeff_k = 1 / sqrt(64) / sqrt(mul)

Strategy (8 NeuronCores, SPMD single program):
- Row-shard: core c owns rows [1024c, 1024(c+1)).
- Host splits each core's rows into segment-pure chunks of <=128 rows, padded
  to exactly 128 (zeros), NCH chunks per core (max over cores, dummy-padded).
- Host packs inputs TRANSPOSED (contraction dim m on partitions) so all device
  DMAs are contiguous, 128-partition:
    in0[m, k*128 + r]                      = x0[row, m]
    in1[(k%2)*64 + m, (k//2)*384 + i*128 + r] = x1[row, m, i]   (pairs share cols)
    in2[(k%4)*32 + m, (k//4)*640 + i*128 + r] = x2[row, m, i]   (quads share cols)
  Weights gathered per chunk, laid out as lhsT (m, o) at matching partitions.
- Device: per chunk: matmul irrep0 (K=128,M=128,N=128), irrep1 (K=64,M=64,
  N=384, tile_position parity packing), irrep2 (K=32,M=32,N=512+128, quad
  packing).  PSUM evicted with copy*coeff to packed SBUF out images, DMA'd out.
- Host unpacks (inverse transpose) to the reference output layout.
"""

import math

import numpy as np

IRREPS = [(128, 1), (64, 3), (32, 5)]
NUM_IDX = 64
N_EL = 8192
NCORES = 8
RPC = N_EL // NCORES  # rows per core
C = 128  # chunk rows
W_OFF = [0, 128 * 128, 128 * 128 + 64 * 64]
COEFF = [1.0 / (8.0 * math.sqrt(128.0)),
         1.0 / (8.0 * math.sqrt(64.0)),
         1.0 / (8.0 * math.sqrt(32.0))]


def _plan(counts):
    """Per-core list of (segment, src_row, real_len) chunks, padded to common NCH."""
    starts = np.zeros(NUM_IDX + 1, np.int64)
    starts[1:] = np.cumsum(counts.astype(np.int64))
    assert starts[-1] == N_EL, f"counts sum {starts[-1]} != {N_EL}"
    per_core = []
    for c in range(NCORES):
        lo_c, hi_c = c * RPC, (c + 1) * RPC
        chunks = []
        for g in range(NUM_IDX):
            a = max(int(starts[g]), lo_c)
            b = min(int(starts[g + 1]), hi_c)
            if a >= b:
                continue
            for s in range(a, b, C):
                chunks.append((g, s, min(C, b - s)))
        per_core.append(chunks)
    nch = max(len(ch) for ch in per_core)
    nch = max(nch, 1)
    for ch in per_core:
        ch.extend([(0, 0, 0)] * (nch - len(ch)))
    return per_core, nch


def _pack_core(chunks, nch, x0, x1, x2, w):
    """Build the six packed device input arrays for one core."""
    p1 = (nch + 1) // 2
    p2 = (nch + 3) // 4
    in0 = np.zeros((128, nch * 128), np.float32)
    in1 = np.zeros((128, p1 * 384), np.float32)
    in2 = np.zeros((128, p2 * 640), np.float32)
    w0 = np.zeros((128, nch * 128), np.float32)
    w1 = np.zeros((128, p1 * 128), np.float32)
    w2 = np.zeros((128, p2 * 128), np.float32)
    for k, (g, s, ln) in enumerate(chunks):
        if ln > 0:
            in0[:, k * 128:k * 128 + ln] = x0[s:s + ln, :, 0].T
            # x1 block: (ln, 64, 3) -> (m, i, r)
            b1 = x1[s:s + ln].transpose(1, 2, 0)
            h = (k % 2) * 64
            c1 = (k // 2) * 384
            in1[h:h + 64, c1:c1 + 384].reshape(64, 3, 128)[:, :, :ln] = b1
            b2 = x2[s:s + ln].transpose(1, 2, 0)
            h2 = (k % 4) * 32
            c2 = (k // 4) * 640
            in2[h2:h2 + 32, c2:c2 + 640].reshape(32, 5, 128)[:, :, :ln] = b2
            w0[:, k * 128:(k + 1) * 128] = w[g, W_OFF[0]:W_OFF[0] + 128 * 128].reshape(128, 128)
            h1 = (k % 2) * 64
            w1[h1:h1 + 64, (k // 2) * 128 + h1:(k // 2) * 128 + h1 + 64] = \
                w[g, W_OFF[1]:W_OFF[1] + 64 * 64].reshape(64, 64)
            h2b = (k % 4) * 32
            w2[h2b:h2b + 32, (k // 4) * 128 + h2b:(k // 4) * 128 + h2b + 32] = \
                w[g, W_OFF[2]:W_OFF[2] + 32 * 32].reshape(32, 32)
    return {"in0": in0, "in1": in1, "in2": in2, "w0": w0, "w1": w1, "w2": w2}


def _unpack_core(chunks, outs, y0, y1, y2):
    out0, out1, out2 = outs["out0"], outs["out1"], outs["out2"]
    for k, (g, s, ln) in enumerate(chunks):
        if ln <= 0:
            continue
        y0[s:s + ln, :, 0] = out0[:, k * 128:k * 128 + ln].T
        h = (k % 2) * 64
        c1 = (k // 2) * 384
        # out1[h+o, c1 + i*128 + r] -> y1[s+r, o, i]
        blk = out1[h:h + 64, c1:c1 + 384].reshape(64, 3, 128)[:, :, :ln]
        y1[s:s + ln] = blk.transpose(2, 0, 1)
        h2 = (k % 4) * 32
        c2 = (k // 4) * 640
        blk2 = out2[h2:h2 + 32, c2:c2 + 640].reshape(32, 5, 128)[:, :, :ln]
        y2[s:s + ln] = blk2.transpose(2, 0, 1)


def _emulate_core(ins, nch):
    """Numpy emulation of the device program (for logic validation)."""
    p1 = (nch + 1) // 2
    p2 = (nch + 3) // 4
    out0 = np.zeros((128, nch * 128), np.float32)
    out1 = np.zeros((128, p1 * 384), np.float32)
    out2 = np.zeros((128, p2 * 640), np.float32)
    for k in range(nch):
        lhsT = ins["w0"][:, k * 128:(k + 1) * 128]
        rhs = ins["in0"][:, k * 128:(k + 1) * 128]
        out0[:, k * 128:(k + 1) * 128] = COEFF[0] * (lhsT.T @ rhs)
    for jp in range(p1):
        lhsT = ins["w1"][:, jp * 128:(jp + 1) * 128]
        rhs = ins["in1"][:, jp * 384:(jp + 1) * 384]
        out1[:, jp * 384:(jp + 1) * 384] = COEFF[1] * (lhsT.T @ rhs)
    for qq in range(p2):
        lhsT = ins["w2"][:, qq * 128:(qq + 1) * 128]
        rhs = ins["in2"][:, qq * 640:(qq + 1) * 640]
        out2[:, qq * 640:(qq + 1) * 640] = COEFF[2] * (lhsT.T @ rhs)
    return {"out0": out0, "out1": out1, "out2": out2}


# ---------------------------------------------------------------- device ---

_PROGRAM_CACHE = {}
LAST_RESULTS = None  # BassKernelResults of the most recent device run


def _build_program(nch):
    import concourse.mybir as mybir
    import concourse.tile as tile
    from concourse import bacc

    f32 = mybir.dt.float32
    f32r = mybir.dt.float32r  # single-pass fp32 matmul (1 cyc/col at N>=256)
    p1 = (nch + 1) // 2
    p2 = (nch + 3) // 4

    nc = bacc.Bacc(None, target_bir_lowering=False)
    d_in0 = nc.dram_tensor("in0", [128, nch * 128], f32r, kind="ExternalInput")
    d_in1 = nc.dram_tensor("in1", [128, p1 * 384], f32r, kind="ExternalInput")
    d_in2 = nc.dram_tensor("in2", [128, p2 * 640], f32r, kind="ExternalInput")
    d_w0 = nc.dram_tensor("w0", [128, nch * 128], f32r, kind="ExternalInput")
    d_w1 = nc.dram_tensor("w1", [128, p1 * 128], f32r, kind="ExternalInput")
    d_w2 = nc.dram_tensor("w2", [128, p2 * 128], f32r, kind="ExternalInput")
    d_out0 = nc.dram_tensor("out0", [128, nch * 128], f32, kind="ExternalOutput")
    d_out1 = nc.dram_tensor("out1", [128, p1 * 384], f32, kind="ExternalOutput")
    d_out2 = nc.dram_tensor("out2", [128, p2 * 640], f32, kind="ExternalOutput")

    n_pieces = p2  # one piece = up to 4 chunks (2 pairs, 1 quad)

    def piece_chunks(q):
        return range(q * 4, min((q + 1) * 4, nch))

    with tile.TileContext(nc) as tc:
        with (
            tc.tile_pool(name="sb_in", bufs=1) as sb_in,
            tc.tile_pool(name="sb_out", bufs=1) as sb_out,
            tc.tile_pool(name="psA", bufs=2, space="PSUM") as psA,
        ):
            t_in0, t_in1, t_in2, t_w0, t_w1, t_w2 = [], [], [], [], [], []
            # input DMAs, piece-interleaved so early pieces complete first
            for q in range(n_pieces):
                ch = list(piece_chunks(q))
                nch_q = len(ch)
                npair_q = (nch_q + 1) // 2
                a0 = sb_in.tile([128, nch_q * 128], f32r, tag=f"in0_{q}")
                nc.sync.dma_start(a0[:], d_in0[:, q * 512:q * 512 + nch_q * 128])
                a1 = sb_in.tile([128, npair_q * 384], f32r, tag=f"in1_{q}")
                nc.sync.dma_start(a1[:], d_in1[:, q * 768:q * 768 + npair_q * 384])
                a2 = sb_in.tile([128, 640], f32r, tag=f"in2_{q}")
                nc.sync.dma_start(a2[:], d_in2[:, q * 640:(q + 1) * 640])
                b0 = sb_in.tile([128, nch_q * 128], f32r, tag=f"w0_{q}")
                nc.sync.dma_start(b0[:], d_w0[:, q * 512:q * 512 + nch_q * 128])
                b1 = sb_in.tile([128, npair_q * 128], f32r, tag=f"w1_{q}")
                nc.sync.dma_start(b1[:], d_w1[:, q * 256:q * 256 + npair_q * 128])
                b2 = sb_in.tile([128, 128], f32r, tag=f"w2_{q}")
                nc.sync.dma_start(b2[:], d_w2[:, q * 128:(q + 1) * 128])
                t_in0.append(a0); t_in1.append(a1); t_in2.append(a2)
                t_w0.append(b0); t_w1.append(b1); t_w2.append(b2)

            for q in range(n_pieces):
                ch = list(piece_chunks(q))
                nch_q = len(ch)
                npair_q = (nch_q + 1) // 2
                # 4 PSUM banks per piece (x2 bufs = all 8 banks):
                #   po0:   irrep0, 4 chunks x 128 cols
                #   po1a:  irrep1 pair0 [0:384] + irrep2 tail cols [384:512]
                #   po1b:  irrep1 pair1
                #   po2a:  irrep2 head (5 i-slices x 128 = 640 -> 512 here)
                po0 = psA.tile([128, nch_q * 128], f32, tag="po0")
                po1a = psA.tile([128, 512], f32, tag="po1a")
                po1 = [po1a[:, 0:384]]
                if npair_q > 1:
                    po1b = psA.tile([128, 384], f32, tag="po1b")
                    po1.append(po1b[:, :])
                po2a = psA.tile([128, 512], f32, tag="po2a")
                po2b = po1a[:, 384:512]
                for j, k in enumerate(ch):
                    # irrep0: K=128, M=128, N=128
                    nc.tensor.matmul(
                        po0[:, j * 128:(j + 1) * 128],
                        t_w0[q][:, j * 128:(j + 1) * 128],
                        t_in0[q][:, j * 128:(j + 1) * 128],
                        start=True, stop=True,
                    )
                # irrep1: per pair, K=128 blockdiag, N=384 (fp32r full rate)
                for jp in range(npair_q):
                    nc.tensor.matmul(
                        po1[jp][:, :],
                        t_w1[q][:, jp * 128:(jp + 1) * 128],
                        t_in1[q][:, jp * 384:(jp + 1) * 384],
                        start=True, stop=True,
                    )
                # irrep2: per quad, K=128 blockdiag, N=512+128
                nc.tensor.matmul(
                    po2a[:, :],
                    t_w2[q][:, :],
                    t_in2[q][:, 0:512],
                    start=True, stop=True,
                )
                nc.tensor.matmul(
                    po2b[:, :],
                    t_w2[q][:, :],
                    t_in2[q][:, 512:640],
                    start=True, stop=True,
                )
                # evictions (copy * coeff) into packed out tiles
                o0 = sb_out.tile([128, nch_q * 128], f32, tag="o0")
                nc.any.tensor_scalar_mul(o0[:], po0[:], COEFF[0])
                o1 = sb_out.tile([128, npair_q * 384], f32, tag="o1")
                for jp in range(npair_q):
                    nc.any.tensor_scalar_mul(
                        o1[:, jp * 384:(jp + 1) * 384], po1[jp][:], COEFF[1])
                o2 = sb_out.tile([128, 640], f32, tag="o2")
                nc.any.tensor_scalar_mul(o2[:, 0:512], po2a[:], COEFF[2])
                nc.any.tensor_scalar_mul(o2[:, 512:640], po2b[:], COEFF[2])
                nc.sync.dma_start(d_out0[:, q * 512:q * 512 + nch_q * 128], o0[:])
                nc.sync.dma_start(d_out1[:, q * 768:q * 768 + npair_q * 384], o1[:])
                nc.sync.dma_start(d_out2[:, q * 640:(q + 1) * 640], o2[:])

    nc.compile()
    return nc


def _run_device(per_core_ins, nch):
    global LAST_RESULTS
    from concourse.bass_utils import run_bass_kernel_spmd

    if nch not in _PROGRAM_CACHE:
        _PROGRAM_CACHE[nch] = _build_program(nch)
    nc = _PROGRAM_CACHE[nch]
    res = run_bass_kernel_spmd(nc, per_core_ins, core_ids=list(range(NCORES)))
    LAST_RESULTS = res
    return res.results


def kernel(x0, x1, x2, w, num_index_counts, _emulate=False):
    x0 = np.asarray(x0, np.float32)
    x1 = np.asarray(x1, np.float32)
    x2 = np.asarray(x2, np.float32)
    w = np.asarray(w, np.float32)
    counts = np.asarray(num_index_counts)
    per_core, nch = _plan(counts)
    per_core_ins = [_pack_core(ch, nch, x0, x1, x2, w) for ch in per_core]
    if _emulate:
        per_core_outs = [_emulate_core(ins, nch) for ins in per_core_ins]
    else:
        per_core_outs = _run_device(per_core_ins, nch)
    y0 = np.zeros((N_EL, 128, 1), np.float32)
    y1 = np.zeros((N_EL, 64, 3), np.float32)
    y2 = np.zeros((N_EL, 32, 5), np.float32)
    for ch, outs in zip(per_core, per_core_outs):
        _unpack_core(ch, outs, y0, y1, y2)
    return y0, y1, y2
